# revision 7
# baseline (speedup 1.0000x reference)
"""Trainium2 Bass kernel for nn_BatchCropElements: out = x * (rand_u > 0.3).

Full inputs: x [64, 2048, 24, 12] f32, rand_u [24, 12] f32.
Sharding: data-parallel on batch across 8 cores -> per-core 8*2048 = 16384
spatial planes of 288 f32 (24*12), streamed HBM -> SBUF -> multiply by the
0/1 mask -> HBM.

Per-core schedule (builder "skew"):
- SDMA engine 15 (serving SBUF partitions 92-95 and 124-127) runs ~15-18%
  slower than engines 0-14 on this part (measured 22.5 vs 26.4 GB/s busy
  rate, per-packet 820 vs 697 ns for identical 18 KB packets). DMA work is
  statically split by partition, so a uniform layout makes engine 15 the
  ~10 us long pole. The host therefore packs 130 planes into each of the
  120 "fast" partitions and only 98 into the 8 engine-15 partitions.
- The mask ride loads first on the Sync HWDGE ring so it lands ~8 us in;
  DVE thresholds + log-doubles it to one chunk width.
- 14 tapered chunks (10-plane steady state, 6+4 at the end) keep the
  serial endgame (last load -> last mul -> last store) small.
- Raw engine blocks with per-chunk semaphores (no Tile): loads on Sync,
  muls on DVE, stores on ACT, one final wait per store sem. No semaphore
  reuse -> no issue stalls.
"""

from contextlib import ExitStack

import numpy as np

import concourse.bass as bass
import concourse.tile as tile
from concourse import bacc, mybir
from concourse.bass_utils import run_bass_kernel_spmd

N_CORES = 8
B, C, H, W = 64, 2048, 24, 12
HW = H * W  # 288
B_SH = B // N_CORES  # 8 batches per core
P = 128
PLANES = B_SH * C  # 16384 spatial planes per core
PROB = 0.3

_DT = mybir.dt.float32

# ---- skewed layout ----------------------------------------------------------
# HWDGE splits a DMA's partition dim across d = (largest divisor of the
# partition count <= 16) SDMA engine slots, contiguous row blocks, starting
# at slot 0. So 128-row DMAs put rows 120-127 on engine 15, and 120-row
# DMAs (120 = 15 x 8) engage exactly engines 0-14 with 8 rows each.
# Layout: one padded DRAM tensor [128, 130*288]; rows 120-127 (engine 15)
# hold only 98 planes, rows 0-119 hold 130. Phase A (planes 0..98) streams
# 128-row chunks; phase B (planes 98..130) streams 120-row chunks that
# skip engine 15 entirely.
P_FAST = 129  # planes per fast partition (rows 0-119)
P_SLOW = 113  # planes per engine-15 partition (rows 120-127)
assert 120 * P_FAST + 8 * P_SLOW == PLANES
F_FAST = P_FAST * HW  # f32 per fast row (padded row length)
F_SLOW = P_SLOW * HW  # f32 valid in slow rows

# wider chunks: >=14-plane rows (16128B packets) keep per-engine rate high
AW = [15] + [14] * 7  # phase A chunk widths (planes), 128 rows each
BW = [12, 4]  # phase B chunk widths (planes), 120 rows each
assert sum(AW) == P_SLOW and sum(BW) == P_FAST - P_SLOW
N_CHUNK = len(AW) + len(BW)
WMAX = max(AW + BW) * HW


def _build_nc_skew() -> bass.Bass:
    nc = bacc.Bacc()
    x = nc.declare_dram_parameter("x", [P, F_FAST], _DT, isOutput=False)
    u = nc.declare_dram_parameter("u", [P, HW], _DT, isOutput=False)
    out = nc.declare_dram_parameter("out", [P, F_FAST], _DT, isOutput=True)

    # chunk table: (col_start, col_end, n_rows)
    chunks = []
    pos = 0
    for w in AW:
        chunks.append((pos * HW, (pos + w) * HW, P))
        pos += w
    for w in BW:
        chunks.append((pos * HW, (pos + w) * HW, 120))
        pos += w
    assert pos == P_FAST

    with ExitStack() as ctx:
        tu = ctx.enter_context(nc.sbuf_tensor("tu", [P, HW], _DT))
        bmask = ctx.enter_context(nc.sbuf_tensor("bmask", [P, WMAX], _DT))
        ts = [
            ctx.enter_context(nc.sbuf_tensor(f"t{c}", [P, b - a], _DT))
            for c, (a, b, _) in enumerate(chunks)
        ]
        msem = ctx.enter_context(nc.semaphore("msem"))
        mk_sem = ctx.enter_context(nc.semaphore("mk"))
        mul_sem = ctx.enter_context(nc.semaphore("mul"))
        ld_sems = [
            ctx.enter_context(nc.semaphore(f"ld{c}")) for c in range(N_CHUNK)
        ]
        st_sems = [
            ctx.enter_context(nc.semaphore(f"st{c}")) for c in range(N_CHUNK)
        ]
        block = ctx.enter_context(nc.Block())

        @block.sync
        def _(sync):
            # mask first: its 128 tiny packets interleave ahead of the bulk
            sync.dma_start(out=tu[:], in_=u[:, :]).then_inc(msem, 16)
            for c, (a, b, rows) in enumerate(chunks):
                sync.dma_start(
                    out=ts[c][0:rows, :], in_=x[0:rows, a:b]
                ).then_inc(ld_sems[c], 16)

        @block.vector
        def _(vector):
            # DVE is pipelined: same-engine RAW chains need explicit sems.
            vector.wait_ge(msem, 16)
            vector.tensor_scalar(
                out=bmask[:, 0:HW],
                in0=tu[:],
                scalar1=PROB,
                scalar2=None,
                op0=mybir.AluOpType.is_gt,
            ).then_inc(mk_sem, 1)
            n_mk = 1
            w = HW
            while w < WMAX:
                cp = min(w, WMAX - w)
                vector.wait_ge(mk_sem, n_mk)
                vector.tensor_copy(
                    out=bmask[:, w : w + cp], in_=bmask[:, 0:cp]
                ).then_inc(mk_sem, 1)
                w += cp
                n_mk += 1
            for c, (a, b, rows) in enumerate(chunks):
                if c == 0:
                    vector.wait_ge(mk_sem, n_mk)
                vector.wait_ge(ld_sems[c], 16)
                vector.tensor_tensor(
                    out=ts[c][0:rows, :],
                    in0=ts[c][0:rows, :],
                    in1=bmask[0:rows, 0 : b - a],
                    op=mybir.AluOpType.mult,
                ).then_inc(mul_sem, 1)

        @block.scalar
        def _(scalar):
            for c, (a, b, rows) in enumerate(chunks):
                scalar.wait_ge(mul_sem, c + 1)
                scalar.dma_start(
                    out=out[0:rows, a:b], in_=ts[c][0:rows, :]
                ).then_inc(st_sems[c], 16)

        # Final store-completion waits live on the otherwise-idle PE array
        # engine: it runs its ~51 epilogue semaphore resets early (during the
        # stream), so the NEFF end barrier follows the last store's landing
        # almost immediately; ACT's own resets overlap the final flight.
        @block.tensor
        def _(tensor):
            for c in range(N_CHUNK):
                tensor.wait_ge(st_sems[c], 16)

    nc.finalize()
    return nc


# ---- uniform raw variant (same scaffolding, no engine-15 skew) --------------
UF_TOTAL = PLANES // P  # 128 planes per partition
UFW = [16] * 7 + [12, 4]
assert sum(UFW) == UF_TOTAL


def _build_nc_rawu() -> bass.Bass:
    nc = bacc.Bacc()
    x = nc.declare_dram_parameter("x", [P, UF_TOTAL * HW], _DT, isOutput=False)
    u = nc.declare_dram_parameter("u", [P, HW], _DT, isOutput=False)
    out = nc.declare_dram_parameter("out", [P, UF_TOTAL * HW], _DT, isOutput=True)
    n = len(UFW)
    cf = [sum(UFW[:i]) * HW for i in range(n + 1)]
    wmax = max(UFW) * HW

    with ExitStack() as ctx:
        tu = ctx.enter_context(nc.sbuf_tensor("tu", [P, HW], _DT))
        bmask = ctx.enter_context(nc.sbuf_tensor("bmask", [P, wmax], _DT))
        ts = [
            ctx.enter_context(nc.sbuf_tensor(f"t{c}", [P, UFW[c] * HW], _DT))
            for c in range(n)
        ]
        msem = ctx.enter_context(nc.semaphore("msem"))
        mk_sem = ctx.enter_context(nc.semaphore("mk"))
        mul_sem = ctx.enter_context(nc.semaphore("mul"))
        ld_sems = [ctx.enter_context(nc.semaphore(f"ld{c}")) for c in range(n)]
        st_sems = [ctx.enter_context(nc.semaphore(f"st{c}")) for c in range(n)]
        block = ctx.enter_context(nc.Block())

        @block.sync
        def _(sync):
            sync.dma_start(out=tu[:], in_=u[:, :]).then_inc(msem, 16)
            for c in range(n):
                sync.dma_start(
                    out=ts[c][:], in_=x[:, cf[c] : cf[c + 1]]
                ).then_inc(ld_sems[c], 16)

        @block.vector
        def _(vector):
            vector.wait_ge(msem, 16)
            vector.tensor_scalar(
                out=bmask[:, 0:HW],
                in0=tu[:],
                scalar1=PROB,
                scalar2=None,
                op0=mybir.AluOpType.is_gt,
            ).then_inc(mk_sem, 1)
            n_mk = 1
            w = HW
            while w < wmax:
                cp = min(w, wmax - w)
                vector.wait_ge(mk_sem, n_mk)
                vector.tensor_copy(
                    out=bmask[:, w : w + cp], in_=bmask[:, 0:cp]
                ).then_inc(mk_sem, 1)
                w += cp
                n_mk += 1
            for c in range(n):
                if c == 0:
                    vector.wait_ge(mk_sem, n_mk)
                fw = UFW[c] * HW
                vector.wait_ge(ld_sems[c], 16)
                vector.tensor_tensor(
                    out=ts[c][:],
                    in0=ts[c][:],
                    in1=bmask[:, 0:fw],
                    op=mybir.AluOpType.mult,
                ).then_inc(mul_sem, 1)

        @block.scalar
        def _(scalar):
            for c in range(n):
                scalar.wait_ge(mul_sem, c + 1)
                scalar.dma_start(
                    out=out[:, cf[c] : cf[c + 1]], in_=ts[c][:]
                ).then_inc(st_sems[c], 16)
            for c in range(n):
                scalar.wait_ge(st_sems[c], 16)

    nc.finalize()
    return nc


# ---- previous-best Tile variant (hw8) for fallback/A-B ----------------------
F_TOTAL = PLANES * HW // P  # 36864
F_HW8 = 4608


def _build_nc_hw8() -> bass.Bass:
    n_chunk = F_TOTAL // F_HW8
    nc = bacc.Bacc()
    x = nc.declare_dram_parameter("x", [P, F_TOTAL], _DT, isOutput=False)
    u = nc.declare_dram_parameter("u", [P, HW], _DT, isOutput=False)
    out = nc.declare_dram_parameter("out", [P, F_TOTAL], _DT, isOutput=True)

    with tile.TileContext(nc) as tc:
        with (
            tc.tile_pool(name="upool", bufs=1) as upool,
            tc.tile_pool(name="maskp", bufs=1) as maskp,
            tc.tile_pool(name="iop", bufs=n_chunk) as iop,
        ):
            tu = upool.tile([P, HW], _DT)
            nc.scalar.dma_start(out=tu[:], in_=u[:, :])
            bmask = maskp.tile([P, F_HW8], _DT)
            nc.vector.tensor_scalar(
                out=bmask[:, 0:HW],
                in0=tu[:],
                scalar1=PROB,
                scalar2=None,
                op0=mybir.AluOpType.is_gt,
            )
            w = HW
            while w < F_HW8:
                nc.vector.tensor_copy(out=bmask[:, w : 2 * w], in_=bmask[:, 0:w])
                w *= 2
            for c in range(n_chunk):
                t = iop.tile([P, F_HW8], _DT, name="t")
                nc.sync.dma_start(out=t[:], in_=x[:, c * F_HW8 : (c + 1) * F_HW8])
                nc.vector.tensor_mul(out=t[:], in0=t[:], in1=bmask[:])
                nc.scalar.dma_start(
                    out=out[:, c * F_HW8 : (c + 1) * F_HW8], in_=t[:]
                )
    nc.finalize()
    return nc


BUILDER = "skew"
_NC_CACHE: dict = {}


def _get_nc(key: str):
    if key not in _NC_CACHE:
        _NC_CACHE[key] = {
            "skew": _build_nc_skew,
            "rawu": _build_nc_rawu,
            "hw8": _build_nc_hw8,
        }[key]()
    return _NC_CACHE[key]


def _run(inputs: dict, trace: bool = False):
    x = np.ascontiguousarray(inputs["x"], dtype=np.float32)
    rand_u = np.ascontiguousarray(inputs["rand_u"], dtype=np.float32)
    assert x.shape == (B, C, H, W), x.shape
    assert rand_u.shape == (H, W), rand_u.shape

    u_rep = np.ascontiguousarray(
        np.broadcast_to(rand_u.reshape(1, HW), (P, HW)), dtype=np.float32
    )

    nc = _get_nc(BUILDER)
    in_maps = []
    n_fast = 120 * F_FAST  # plane split point in the flat shard
    if BUILDER == "skew":
        for i in range(N_CORES):
            flat = x[i * B_SH : (i + 1) * B_SH].reshape(-1)
            xall = np.zeros((P, F_FAST), dtype=np.float32)
            xall[:120] = flat[:n_fast].reshape(120, F_FAST)
            xall[120:, :F_SLOW] = flat[n_fast:].reshape(8, F_SLOW)
            in_maps.append({"x": xall, "u": u_rep})
    else:
        for i in range(N_CORES):
            shard = x[i * B_SH : (i + 1) * B_SH].reshape(P, F_TOTAL)
            in_maps.append({"x": shard, "u": u_rep})

    res = run_bass_kernel_spmd(nc, in_maps, list(range(N_CORES)), trace=trace)
    out = np.empty((B, C, H, W), dtype=np.float32)
    for i in range(N_CORES):
        r = res.results[i]
        if BUILDER == "skew":
            o = r["out"]
            flat = np.concatenate(
                [o[:120].reshape(-1), o[120:, :F_SLOW].reshape(-1)]
            )
            out[i * B_SH : (i + 1) * B_SH] = flat.reshape(B_SH, C, H, W)
        else:
            out[i * B_SH : (i + 1) * B_SH] = r["out"].reshape(B_SH, C, H, W)
    return out, res


def kernel(**inputs: np.ndarray) -> np.ndarray:
    out, _ = _run(inputs, trace=False)
    return out
